# revision 1
# baseline (speedup 1.0000x reference)
"""Multi-head attention (B=2, S=2048, D=1024, H=16, causal + key/query masks)
on 8 Trainium2 NeuronCores.

Sharding: head-parallel. Core c owns heads {2c, 2c+1}: Wq/Wk/Wv column-sliced,
Wo row-sliced. q/k replicated (each core projects all tokens onto its 2 heads).
Partial outputs (through the Wo row-slice) are summed on the host; bo and the
q_mask row-zeroing (those rows equal bo exactly) are applied on the host.

Device dataflow per core (big matmuls in float32r — full PE rate, ~1e-4 rel):
  per batch b:
  - QhT/KhT [128feat, 512tok] group tiles: projections in transposed layout
    (features on partitions), contraction over D via 8 accumulating matmuls;
    inputs host-pre-arranged so each chunk DMA is one contiguous run per
    partition. Small per-group tiles keep Tile's per-tensor deps fine-grained.
  - Vh [tok, feat(+ones)] via PE transpose of V^T chunks; the appended ones
    column makes the AV matmul emit softmax denominators for free.
  - scores^T tiles [128 k-tok, q-range] = KhT^T @ QhT, the 2 heads row-tiled
    on the PE; exp on ACT with fused 1/8 scale + k_mask(-1e30) per-partition
    bias; causal via restricted q-ranges + a 0/1 triangular multiply on the
    diagonal block (half the flops/exp skipped).
  - P^T tiles feed AV matmuls directly as the moving operand (no transposes).
  - row-sum broadcast via a rank-1 PE matmul (gpsimd partition_broadcast
    reads the wrong partition on HW for base>0 sources — avoid), reciprocal
    + normalize on DVE, Wo partial projection, evacuate via ACT, and DMA out
    partialT [1024 feat, 4096 tok] on the second HWDGE ring (ACT) so output
    stores never block streaming input loads on the SP ring.
"""
import os
import numpy as np

B, S, D, H = 2, 2048, 1024, 16
NT = B * S            # 4096 tokens
NCORES = 8
HPC = H // NCORES     # heads per core = 2
TCH = int(os.environ.get("KTCH", "256"))  # projection token-chunk
QB = 512              # attention q-chunk
NKB = S // 128        # 16 k-blocks per batch
NEG = -1.0e30

_CACHE = {}
LAST_EXEC_TIME_NS = None
LAST_RESULTS = None


def _build(causal: bool, reps: int = 1, tch: int = None):
    import concourse.bass as bass  # noqa: F401
    from concourse import bacc
    import concourse.tile as tile
    import concourse.mybir as mybir
    from concourse.masks import make_identity

    dt = mybir.dt
    f32, f32r = dt.float32, dt.float32r
    TCH = tch if tch is not None else globals()["TCH"]

    nc = bacc.Bacc("TRN2", target_bir_lowering=False, debug=False,
                   num_devices=NCORES)

    # host pre-arranged: [tch, p, kc*TCH] so each chunk load is one
    # contiguous run per partition (128 descriptors instead of 1024)
    qT = nc.dram_tensor("qT", [NT // TCH, 128, 8 * TCH], f32r,
                        kind="ExternalInput")
    kT = nc.dram_tensor("kT", [NT // TCH, 128, 8 * TCH], f32r,
                        kind="ExternalInput")
    wqkv = nc.dram_tensor("wqkv", [128, 8 * 384], f32r, kind="ExternalInput")
    bqkv = nc.dram_tensor("bqkv", [128, 3], f32, kind="ExternalInput")
    wo2 = nc.dram_tensor("wo2", [64, 2 * D], f32r, kind="ExternalInput")
    kmaskT = nc.dram_tensor("kmaskT", [128, B * NKB], f32, kind="ExternalInput")
    partialT = nc.dram_tensor("partialT", [D, NT], f32, kind="ExternalOutput")

    with tile.TileContext(nc) as tc:
        with tc.tile_pool(name="const", bufs=1) as constp, \
             tc.tile_pool(name="proj", bufs=8) as projp, \
             tc.tile_pool(name="qin", bufs=int(os.environ.get("KQIN", "5" if TCH == 256 else "2"))) as qinp, \
             tc.tile_pool(name="vtmp", bufs=2) as vtmpp, \
             tc.tile_pool(name="pt", bufs=int(os.environ.get("KPT", "3"))) as ptp, \
             tc.tile_pool(name="attn65", bufs=int(os.environ.get("KA65B", "5"))) as a65p, \
             tc.tile_pool(name="anorm", bufs=3) as anp, \
             tc.tile_pool(name="scale", bufs=2) as scalep, \
             tc.tile_pool(name="outs", bufs=int(os.environ.get("KOUTB", "3"))) as outp, \
             tc.tile_pool(name="psmisc", bufs=int(os.environ.get("KPSM", "2")), space="PSUM") as psmisc, \
             tc.tile_pool(name="pss", bufs=2, space="PSUM") as pss, \
             tc.tile_pool(name="psav", bufs=int(os.environ.get("KPSAV", "2")), space="PSUM") as psav:

            # ---- constants / weights ----
            ident = constp.tile([128, 128], f32, tag="ident")
            make_identity(nc, ident[:])
            # tri01[kk, qq] = 1.0 if qq >= kk else 0.0 (keep-causal multiplier)
            tri01 = constp.tile([128, 128], f32, tag="tri01")
            nc.vector.memset(tri01[:], 1.0)
            nc.gpsimd.affine_select(
                out=tri01[:], in_=tri01[:],
                compare_op=mybir.AluOpType.is_ge, fill=0.0, base=0,
                pattern=[[1, 128]], channel_multiplier=-1)
            ones64 = constp.tile([128, 64], f32, tag="ones64")
            nc.vector.memset(ones64[:], 1.0)
            ones64r = constp.tile([128, 64], f32r, tag="ones64r")
            nc.vector.tensor_copy(ones64r[:], ones64[:])

            wqkv_sb = constp.tile([128, 8, 384], f32r, tag="wqkv")
            nc.sync.dma_start(wqkv_sb[:],
                              wqkv.rearrange("p (kc c) -> p kc c", kc=8))
            bqkv_sb = constp.tile([128, 3], f32, tag="bqkv")
            nc.sync.dma_start(bqkv_sb[:], bqkv[:, :])
            wo2_sb = constp.tile([64, 2 * D], f32r, tag="wo2")
            nc.sync.dma_start(wo2_sb[:], wo2[:, :])
            kmaskT_sb = constp.tile([128, B * NKB], f32, tag="kmaskT")
            nc.sync.dma_start(kmaskT_sb[:], kmaskT[:, :])

            NG = S // QB          # 512-token groups per batch
            CPG = QB // TCH       # projection chunks per group

            for _rep in range(reps):
              tiles = {}   # b -> (QhTg, KhTg, Vhg)

              def alloc_group_tiles(b):
                  QhTg, KhTg, Vhg = [], [], []
                  for g in range(NG):
                      QhT = projp.tile([128, QB], f32r, tag="QhT",
                                       name=f"QhT_{b}_{g}")
                      KhT = projp.tile([128, QB], f32r, tag="KhT",
                                       name=f"KhT_{b}_{g}")
                      # Vh: per 128-token block kk (0..3) and head h: [128,65]
                      # (64 feats + ones col) at free offset (kk*2 + h) * 65
                      Vh = projp.tile([128, (QB // 128) * 2 * 65], f32r,
                                      tag="Vh", name=f"Vh_{b}_{g}")
                      nc.vector.tensor_copy(
                          Vh[:].rearrange("p (tb c) -> p tb c", c=65)
                          [:, :, 64:65],
                          ones64[:, 0:(QB // 128) * 2]
                          .rearrange("p (a o) -> p a o", o=1))
                      QhTg.append(QhT)
                      KhTg.append(KhT)
                      Vhg.append(Vh)
                  return QhTg, KhTg, Vhg

              def proj_group(b, g):
                  QhTg, KhTg, Vhg = tiles[b]
                  for cg in range(CPG):
                      tch = g * CPG + cg
                      tg = b * (S // TCH) + tch                 # global chunk
                      ts_l = slice(cg * TCH, (cg + 1) * TCH)    # within group
                      QhT, KhT, Vh = QhTg[g], KhTg[g], Vhg[g]
                      qin = qinp.tile([128, 8, TCH], f32r, tag="qin")
                      nc.sync.dma_start(
                          qin[:], qT[tg].rearrange("p (kc t) -> p kc t", kc=8))
                      kin = qinp.tile([128, 8, TCH], f32r, tag="kin")
                      nc.sync.dma_start(
                          kin[:], kT[tg].rearrange("p (kc t) -> p kc t", kc=8))

                      for j, (src_, dstT) in enumerate(((qin, QhT),
                                                        (kin, KhT))):
                          ps = psmisc.tile([128, TCH], f32, tag="ps")
                          for kc in range(8):
                              nc.tensor.matmul(
                                  ps[:], wqkv_sb[:, kc, j * 128:(j + 1) * 128],
                                  src_[:, kc, :],
                                  start=(kc == 0), stop=(kc == 7))
                          nc.vector.tensor_scalar_add(dstT[:, ts_l], ps[:],
                                                      bqkv_sb[:, j:j + 1])
                      # V projection (from k), then transpose to [tok, feat]
                      ps = psmisc.tile([128, TCH], f32, tag="ps")
                      for kc in range(8):
                          nc.tensor.matmul(ps[:], wqkv_sb[:, kc, 256:384],
                                           kin[:, kc, :],
                                           start=(kc == 0), stop=(kc == 7))
                      vtmp = vtmpp.tile([128, TCH], f32, tag="vtmp")
                      nc.vector.tensor_scalar_add(vtmp[:], ps[:],
                                                  bqkv_sb[:, 2:3])
                      for half in range(TCH // 128):
                          pvT = psmisc.tile([128, 128], f32, tag="ps")
                          nc.tensor.transpose(
                              pvT[:], vtmp[:, half * 128:(half + 1) * 128],
                              ident[:])
                          kk = cg * (TCH // 128) + half       # block in group
                          dst = Vh[:, kk * 130:(kk + 1) * 130] \
                              .rearrange("p (h f) -> p h f", h=2)[:, :, 0:64]
                          nc.vector.tensor_copy(
                              dst, pvT[:].rearrange("p (h f) -> p h f", h=2))

              def norm_and_wo(b, qc, a65pair):
                  # normalization + Wo partial projection for chunk qc
                  attn_norm = {}
                  for h in range(HPC):
                      a65 = a65pair[h]
                      # broadcast the sums row across 64 partitions with a
                      # rank-1 PE matmul (ones[1,64].T @ sums[1,QB])
                      psb = psmisc.tile([64, QB], f32, tag="ps",
                                        name=f"psb_{b}_{qc}_{h}")
                      nc.tensor.matmul(psb[:], ones64r[64:65, 0:64],
                                       a65[64:65, :], start=True, stop=True)
                      recip64 = scalep.tile([64, QB], f32, tag="recip64")
                      nc.vector.reciprocal(recip64[:], psb[:])
                      an = anp.tile([64, QB], f32r, tag="anorm",
                                    name=f"anorm_{b}_{qc}_{h}")
                      nc.vector.tensor_mul(an[:], a65[0:64, :], recip64[:])
                      attn_norm[h] = an
                  for fb in range(D // 128):
                      po = psmisc.tile([128, QB], f32, tag="ps")
                      for h in range(HPC):
                          nc.tensor.matmul(
                              po[:],
                              wo2_sb[:, h * D + fb * 128:h * D + (fb + 1) * 128],
                              attn_norm[h][:],
                              start=(h == 0), stop=(h == HPC - 1))
                      osb = outp.tile([128, QB], f32, tag="outs")
                      if os.environ.get("KOSB", "act") == "act":
                          nc.scalar.copy(osb[:], po[:])
                      else:
                          nc.vector.tensor_copy(osb[:], po[:])
                      nc.scalar.dma_start(
                          partialT[fb * 128:(fb + 1) * 128,
                                   b * S + qc * QB:b * S + (qc + 1) * QB],
                          osb[:])

              def attn_chunk(b, qc):
                  QhTg, KhTg, Vhg = tiles[b]
                  kb_max = (qc * (QB // 128) + (QB // 128) - 1) if causal \
                      else NKB - 1
                  pav = [psav.tile([65, QB], f32, tag="psav",
                                   name=f"pav_{b}_{qc}_{hh}")
                         for hh in range(HPC)]
                  for kb in range(kb_max + 1):
                      qls = max(0, kb * 128 - qc * QB) if causal else 0
                      kg, kk = kb // (QB // 128), kb % (QB // 128)
                      ps = pss.tile([128, 2 * QB], f32, tag="pss")
                      for h in range(HPC):
                          nc.tensor.matmul(
                              ps[:, h * QB + qls:(h + 1) * QB],
                              KhTg[kg][h * 64:(h + 1) * 64,
                                       kk * 128:(kk + 1) * 128],
                              QhTg[qc][h * 64:(h + 1) * 64, qls:QB],
                              start=True, stop=True)
                      pt = ptp.tile([128, 2 * QB], f32r, tag="pt")
                      kbias = kmaskT_sb[:, b * NKB + kb:b * NKB + kb + 1]
                      if qls == 0 and os.environ.get("KEXP", "one") == "one":
                          nc.scalar.activation(
                              pt[:, 0:2 * QB], ps[:, 0:2 * QB],
                              mybir.ActivationFunctionType.Exp,
                              bias=kbias, scale=0.125)
                      else:
                          for h in range(HPC):
                              rg = slice(h * QB + qls, (h + 1) * QB)
                              nc.scalar.activation(
                                  pt[:, rg], ps[:, rg],
                                  mybir.ActivationFunctionType.Exp,
                                  bias=kbias, scale=0.125)
                      if causal and kb >= qc * (QB // 128):
                          for h in range(HPC):
                              dg = slice(h * QB + qls, h * QB + qls + 128)
                              nc.vector.tensor_mul(pt[:, dg], pt[:, dg],
                                                   tri01[:])
                      off = kk * 130
                      for h in range(HPC):
                          nc.tensor.matmul(
                              pav[h][:, qls:QB],
                              Vhg[kg][:, off + h * 65:off + (h + 1) * 65],
                              pt[:, h * QB + qls:(h + 1) * QB],
                              start=(kb == 0), stop=(kb == kb_max))
                  # evacuate pav -> SBUF (a65, f32r); row 64 = sums
                  a65pair = []
                  for h in range(HPC):
                      a65 = a65p.tile([65, QB], f32r, tag="a65",
                                      name=f"a65_{b}_{qc}_{h}")
                      nc.vector.tensor_copy(a65[:], pav[h][:])
                      a65pair.append(a65)
                  norm_and_wo(b, qc, a65pair)

              # projections for BOTH batches first (input DMA streams without
              # ever waiting on attention), then attention for both batches
              # (PE/ACT-dense, output DMA overlapping on the other ring).
              if os.environ.get("KSPLIT", "1") == "1":
                  for b in range(B):
                      tiles[b] = alloc_group_tiles(b)
                      for g in range(NG):
                          proj_group(b, g)
                  for b in range(B):
                      for qc in range(S // QB):
                          attn_chunk(b, qc)
              else:
                  for b in range(B):
                      tiles[b] = alloc_group_tiles(b)
                      for g in range(NG):
                          proj_group(b, g)
                      for qc in range(S // QB):
                          attn_chunk(b, qc)

    nc.compile()
    return nc


def _prep(q, k, k_mask, Wq, bq, Wk, bk, Wv, bv, Wo, tch=None):
    TCH = tch if tch is not None else globals()["TCH"]
    f = np.float32
    q2 = np.asarray(q, dtype=f).reshape(NT, D)
    k2 = np.asarray(k, dtype=f).reshape(NT, D)
    # [tch, p, kc*TCH]: chunk tch, partition p reads one contiguous run
    qTn = np.ascontiguousarray(
        q2.T.reshape(8, 128, NT // TCH, TCH).transpose(2, 1, 0, 3)
        .reshape(NT // TCH, 128, 8 * TCH))
    kTn = np.ascontiguousarray(
        k2.T.reshape(8, 128, NT // TCH, TCH).transpose(2, 1, 0, 3)
        .reshape(NT // TCH, 128, 8 * TCH))
    Wq, Wk, Wv, Wo = (np.asarray(x, dtype=f) for x in (Wq, Wk, Wv, Wo))
    bq, bk, bv = (np.asarray(x, dtype=f) for x in (bq, bk, bv))
    km = np.asarray(k_mask)
    kmaskTn = np.where(km.reshape(B * NKB, 128) == 0, f(NEG), f(0.0))
    kmaskTn = np.ascontiguousarray(kmaskTn.T)  # [128, 32]

    in_maps = []
    for c in range(NCORES):
        hc = slice(128 * c, 128 * (c + 1))
        wqkv_c = np.concatenate([Wq[hc].T, Wk[hc].T, Wv[hc].T], axis=1)
        wqkv_c = np.ascontiguousarray(
            wqkv_c.reshape(8, 128, 384).transpose(1, 0, 2).reshape(128, -1))
        bqkv_c = np.ascontiguousarray(
            np.stack([bq[hc], bk[hc], bv[hc]], axis=1))
        wo2_c = np.ascontiguousarray(np.concatenate(
            [Wo[:, 128 * c:128 * c + 64].T,
             Wo[:, 128 * c + 64:128 * c + 128].T], axis=1))
        in_maps.append({
            "qT": qTn, "kT": kTn, "wqkv": wqkv_c, "bqkv": bqkv_c,
            "wo2": wo2_c, "kmaskT": kmaskTn,
        })
    return in_maps


def kernel(q, k, q_mask, k_mask, Wq, bq, Wk, bk, Wv, bv, Wo, bo,
           causal_attention):
    global LAST_EXEC_TIME_NS, LAST_RESULTS
    from concourse.bass_utils import run_bass_kernel_spmd

    causal = bool(int(np.asarray(causal_attention)))
    if causal not in _CACHE:
        _CACHE[causal] = _build(causal)
    nc = _CACHE[causal]

    in_maps = _prep(q, k, k_mask, Wq, bq, Wk, bk, Wv, bv, Wo)
    trace = os.environ.get("KERNEL_TRACE", "0") == "1"
    try:
        res = run_bass_kernel_spmd(nc, in_maps, list(range(NCORES)),
                                   trace=trace)
    except ModuleNotFoundError:
        # NTFF profiling hook unavailable in this container build
        res = run_bass_kernel_spmd(nc, in_maps, list(range(NCORES)),
                                   trace=False)
    LAST_EXEC_TIME_NS = res.exec_time_ns
    LAST_RESULTS = res

    acc = res.results[0]["partialT"].astype(np.float64)
    for c in range(1, NCORES):
        acc += res.results[c]["partialT"]
    bo32 = np.asarray(bo, dtype=np.float32)
    out = (acc.T.astype(np.float32) + bo32[None, :]).reshape(B, S, D)
    qm0 = np.asarray(q_mask) == 0
    out[qm0] = bo32  # reference: attn rows with q_mask==0 -> out = bo exactly
    return np.ascontiguousarray(out)

